# revision 1
# baseline (speedup 1.0000x reference)
"""Trainium2 Bass kernel for a 3-layer dense transformer (BigramModel).

Contract: kernel(**inputs) takes the FULL unsharded numpy inputs (as produced
by setup_inputs) and returns the full [B, T, V] float32 logits. Internally the
batch dim B=128 is sharded 16-per-core across 8 NeuronCores (pure data
parallelism, weights replicated), one Bass/Tile NEFF run via
run_bass_kernel_spmd.

Layout strategy on device (per core, 16 seqs x 256 tok = 4096 tokens):
  - residual h: token-major fp32 SBUF tiles [128, 384] x 32 (persistent)
  - LayerNorm: DVE bn_stats/bn_aggr per tile; rstd = exp(-0.5*ln(var+eps))
    (keeps ACT in the natural_log_exp table set shared with softmax exp);
    gamma/beta are folded into the adjacent weight matrices on the host.
  - matmuls in bf16 (fp32 PSUM accumulation). Feature-major operands
    (xn^T, o^T) produced by bf16 DMA transposes (XBAR).
  - attention: per (seq, head) scores kept feature-major [s, t] so softmax
    sums run through the matmul path: V is augmented with a ones column so
    the o-matmul also produces the softmax denominators; probs are masked
    multiplicatively after exp (no -inf handling needed).
  - biases that land on free dims (bproj, b2, beta@Wv) are added inside PSUM
    via K=1 ones-row matmuls, emitted only when the host sees nonzero values.
"""

import numpy as np
import ml_dtypes

BF16 = ml_dtypes.bfloat16

P = 128
T = 256
E = 384
V = 65
H = 6
HS = 64
FF = 1536
L = 3
NCORES = 8
BPC = 16              # sequences per core
TOK = BPC * T         # 4096 tokens per core
NT = TOK // P         # 32 token tiles
NB = TOK // 512       # 8 blocks of 512 tokens (2 seqs)
ECH = E // P          # 3
FCH = FF // P         # 12

_NC_CACHE = {}
TMODE = "dma"   # "dma" = XBAR dma transpose; "pe" = TensorE transpose + copy
STAGE = 99      # debug: truncate per-block body (1=LN,2=QKV,3=probs,4=o,5=proj,6=MLP)
SUB = 2         # debug stage-3 detail (legacy)
MLPVAR = "full" # debug: "reuse" skip 2nd LN, "norelu" plain evac, "full"


def _build_nc(flags):
    """Build + compile the Bass program. flags = (bv_nz, bp_nz, b2_nz) per layer."""
    import concourse.bacc as bacc
    import concourse.mybir as mybir
    import concourse.tile as tile

    dt = mybir.dt
    f32 = dt.float32
    bf = dt.bfloat16
    Alu = mybir.AluOpType
    Act = mybir.ActivationFunctionType

    from concourse.masks import make_identity

    nc = bacc.Bacc("TRN2", target_bir_lowering=False, debug=False, num_devices=1)

    # ---- DRAM tensors (shapes match SBUF layouts; host pre-arranges) ----
    D = {}
    D["oh"] = nc.dram_tensor("oh", [V, TOK], bf, kind="ExternalInput")
    D["te"] = nc.dram_tensor("te", [V, E], bf, kind="ExternalInput")
    D["pos"] = nc.dram_tensor("pos", [P, 2, E], f32, kind="ExternalInput")
    D["mask"] = nc.dram_tensor("mask", [P, 2 * P], bf, kind="ExternalInput")
    for l in range(L):
        for w in ("wq", "wk", "wv", "wproj"):
            D[f"{w}{l}"] = nc.dram_tensor(f"{w}{l}", [P, ECH, E], bf, kind="ExternalInput")
        D[f"bq{l}"] = nc.dram_tensor(f"bq{l}", [P, ECH], f32, kind="ExternalInput")
        D[f"bk{l}"] = nc.dram_tensor(f"bk{l}", [P, ECH], f32, kind="ExternalInput")
        D[f"w1{l}"] = nc.dram_tensor(f"w1{l}", [P, ECH, FF], bf, kind="ExternalInput")
        D[f"b1c{l}"] = nc.dram_tensor(f"b1c{l}", [P, FCH], f32, kind="ExternalInput")
        D[f"w2{l}"] = nc.dram_tensor(f"w2{l}", [P, FCH, E], bf, kind="ExternalInput")
        D[f"bvrow{l}"] = nc.dram_tensor(f"bvrow{l}", [1, E], bf, kind="ExternalInput")
        D[f"bpc{l}"] = nc.dram_tensor(f"bpc{l}", [P, ECH], f32, kind="ExternalInput")
        D[f"b2c{l}"] = nc.dram_tensor(f"b2c{l}", [P, ECH], f32, kind="ExternalInput")
    if MLPVAR == "w2dump":
        D["dbga"] = nc.dram_tensor("dbga", [P, FCH, 512], bf, kind="ExternalOutput")
        D["dbgo"] = nc.dram_tensor("dbgo", [P, 4, E], f32, kind="ExternalOutput")
    D["wout"] = nc.dram_tensor("wout", [P, ECH, V], bf, kind="ExternalInput")
    D["boutc"] = nc.dram_tensor("boutc", [V, 1], f32, kind="ExternalInput")
    D["logT"] = nc.dram_tensor("logT", [V, TOK], f32, kind="ExternalOutput")

    bv_nz, bp_nz, b2_nz = flags

    with tile.TileContext(nc) as tc:
        import contextlib

        with contextlib.ExitStack() as ctx:
            const = ctx.enter_context(tc.tile_pool(name="const", bufs=1))
            wpool = ctx.enter_context(tc.tile_pool(name="wpool", bufs=1))
            act = ctx.enter_context(tc.tile_pool(name="act", bufs=3))
            act2 = ctx.enter_context(tc.tile_pool(name="act2", bufs=2))
            act1 = ctx.enter_context(tc.tile_pool(name="act1", bufs=1))
            ps_lin = ctx.enter_context(tc.tile_pool(name="ps_lin", bufs=4, space="PSUM"))
            ps_sc = ctx.enter_context(tc.tile_pool(name="ps_sc", bufs=3, space="PSUM"))

            def load_const(name, shape, dtp):
                t = const.tile(shape, dtp, tag=name)
                nc.sync.dma_start(out=t[:], in_=D[name].ap())
                return t

            # pad the K=65 embedding contraction to K=128 (sub-128 partition
            # matmuls are flaky on HW); pad rows are zeroed so they add 0.
            oh_sb = const.tile([P, TOK], bf, tag="oh")
            nc.vector.memset(oh_sb[:], 0.0)
            nc.sync.dma_start(out=oh_sb[0:V, :], in_=D["oh"].ap())
            te_sb = const.tile([P, E], bf, tag="te")
            nc.vector.memset(te_sb[:], 0.0)
            nc.sync.dma_start(out=te_sb[0:V, :], in_=D["te"].ap())
            pos_sb = load_const("pos", [P, 2, E], f32)
            mask_sb = load_const("mask", [P, 2 * P], bf)
            boutc_sb = load_const("boutc", [V, 1], f32)
            ones_sb = const.tile([1, P], bf, tag="ones")
            nc.vector.memset(ones_sb[:], 1.0)
            eps_sb = const.tile([P, 1], f32, tag="eps")
            nc.vector.memset(eps_sb[:], 1e-5)
            zero_sb = const.tile([P, 1], f32, tag="zero")
            nc.vector.memset(zero_sb[:], 0.0)
            if TMODE == "pe":
                ident_sb = const.tile([P, P], bf, tag="ident")
                make_identity(nc, ident_sb[:])
            _tp_ctr = [0]

            def tpose(dst, src):
                """dst[P,128] (sbuf bf16) = transpose(src[P,128] sbuf bf16)."""
                if TMODE == "dma":
                    nc.sync.dma_start_transpose(dst, src)
                    return
                tp = ps_lin.tile([P, P], f32, tag="mm", name="tp")
                nc.tensor.transpose(tp[:], src, ident_sb[:])
                k = _tp_ctr[0] = _tp_ctr[0] + 1
                if k % 2 == 0:
                    nc.vector.tensor_copy(out=dst, in_=tp[:])
                else:
                    nc.scalar.copy(out=dst, in_=tp[:])

            # persistent residual tiles
            h = []
            for i in range(NT):
                h.append(const.tile([P, E], f32, tag=f"h{i}", name=f"h{i}"))

            # ---- embedding: h = onehot.T @ tok_emb + pos ----
            for i in range(NT):
                ps = ps_lin.tile([P, E], f32, tag="mm")
                nc.tensor.matmul(
                    ps[:], oh_sb[:, i * P:(i + 1) * P], te_sb[:],
                    start=True, stop=True,
                )
                nc.vector.tensor_add(out=h[i][:], in0=ps[:], in1=pos_sb[:, i % 2, :])

            def ln_block(i0, tag):
                """LN of h[i0..i0+3] -> xn bf16 [P,4,E] and xnT bf16 [P,ECH,512]."""
                xn = act2.tile([P, 4, E], bf, tag="xn")
                mv4 = act.tile([P, 4, 2], f32, tag="mv")
                rstd4 = act.tile([P, 4], f32, tag="rstd")
                for j in range(4):
                    st6 = act.tile([P, 6], f32, tag="bnst")
                    nc.vector.bn_stats(out=st6[:], in_=h[i0 + j][:])
                    nc.vector.bn_aggr(out=mv4[:, j, :], in_=st6[:])
                # rstd = exp(-0.5 * ln(var + eps))
                nc.scalar.activation(
                    out=rstd4[:], in_=mv4[:, :, 1], func=Act.Ln, bias=eps_sb[:],
                )
                nc.scalar.activation(
                    out=rstd4[:], in_=rstd4[:], func=Act.Exp, scale=-0.5,
                )
                for j in range(4):
                    nc.vector.tensor_scalar(
                        out=xn[:, j, :], in0=h[i0 + j][:],
                        scalar1=mv4[:, j, 0:1], scalar2=rstd4[:, j:j + 1],
                        op0=Alu.subtract, op1=Alu.mult,
                    )
                xnT = act.tile([P, ECH, 512], bf, tag="xnT")
                for j in range(4):
                    for c in range(ECH):
                        tpose(
                            xnT[:, c, j * P:(j + 1) * P],
                            xn[:, j, c * P:(c + 1) * P],
                        )
                return xnT

            def linear_fmaj(xnT, w_sb, bias_sb, fch, tag, relu=False):
                """feature-major out [P, fch, 512] bf16 = (W^T xn^T); bias per-partition."""
                o = (act1 if fch == FCH else act2).tile([P, fch, 512], bf, tag=tag, name=tag)
                for f in range(fch):
                    ps = ps_lin.tile([P, 512], f32, tag="mm")
                    for c in range(ECH):
                        nc.tensor.matmul(
                            ps[:], w_sb[:, c, f * P:(f + 1) * P], xnT[:, c, :],
                            start=(c == 0), stop=(c == ECH - 1),
                        )
                    if relu:
                        nc.vector.tensor_scalar(
                            out=o[:, f, :], in0=ps[:],
                            scalar1=bias_sb[:, f:f + 1], scalar2=zero_sb[:],
                            op0=Alu.add, op1=Alu.max,
                        )
                    elif bias_sb is not None:
                        nc.vector.tensor_scalar_add(
                            out=o[:, f, :], in0=ps[:], scalar1=bias_sb[:, f:f + 1],
                        )
                    else:
                        nc.vector.tensor_copy(out=o[:, f, :], in_=ps[:])
                return o

            def linear_fmaj_resid(xT, w_sb, nch, bias_col, i0, tag):
                """h[i0+j] += (W^T x)_j via the feature-major matmul pattern
                (weights as lhsT), then DMA-transpose back to token-major."""
                yT = act2.tile([P, ECH, 512], bf, tag="yT", name="yT")
                for f in range(ECH):
                    ps = ps_lin.tile([P, 512], f32, tag="mm")
                    for c in range(nch):
                        nc.tensor.matmul(
                            ps[:], w_sb[:, c, f * P:(f + 1) * P], xT[:, c, :],
                            start=(c == 0), stop=(c == nch - 1),
                        )
                    nc.vector.tensor_scalar_add(
                        out=yT[:, f, :], in0=ps[:], scalar1=bias_col[:, f:f + 1])
                ytm = act2.tile([P, 4, E], bf, tag="ytm", name="ytm")
                for j in range(4):
                    for c in range(ECH):
                        tpose(
                            ytm[:, j, c * P:(c + 1) * P],
                            yT[:, c, j * P:(j + 1) * P],
                        )
                for j in range(4):
                    nc.vector.tensor_add(
                        out=h[i0 + j][:], in0=h[i0 + j][:], in1=ytm[:, j, :])

            def load_w(name, shape, dtp):
                t = wpool.tile(shape, dtp, tag=name[:-1])  # tag without layer idx
                nc.sync.dma_start(out=t[:], in_=D[name].ap())
                return t

            # ---- transformer layers ----
            for l in range(L):
                wq = load_w(f"wq{l}", [P, ECH, E], bf)
                wk = load_w(f"wk{l}", [P, ECH, E], bf)
                wv = load_w(f"wv{l}", [P, ECH, E], bf)
                wproj = load_w(f"wproj{l}", [P, ECH, E], bf)
                bq = load_w(f"bq{l}", [P, ECH], f32)
                bk = load_w(f"bk{l}", [P, ECH], f32)
                w1 = load_w(f"w1{l}", [P, ECH, FF], bf)
                b1c = load_w(f"b1c{l}", [P, FCH], f32)
                w2 = load_w(f"w2{l}", [P, FCH, E], bf)
                bvrow = load_w(f"bvrow{l}", [1, E], bf) if bv_nz[l] else None
                bpc = load_w(f"bpc{l}", [P, ECH], f32)
                b2c = load_w(f"b2c{l}", [P, ECH], f32)

                for b in range(NB):
                    i0 = 4 * b
                    # --- attention sublayer ---
                    if STAGE < 1:
                        continue
                    xnT = ln_block(i0, "a")
                    if STAGE < 2:
                        continue
                    QT = linear_fmaj(xnT, wq, bq, ECH, "QT")
                    KT = linear_fmaj(xnT, wk, bk, ECH, "KT")
                    # V token-major, ones-augmented: [P, 4, H, 65]
                    Vt = act2.tile([P, 4, H, 65], bf, tag="Vt")
                    for j in range(4):
                        ps = ps_lin.tile([P, E], f32, tag="mm")
                        for c in range(ECH):
                            nc.tensor.matmul(
                                ps[:], xnT[:, c, j * P:(j + 1) * P], wv[:, c, :],
                                start=(c == 0),
                                stop=(c == ECH - 1 and bvrow is None),
                            )
                        if bvrow is not None:
                            nc.tensor.matmul(
                                ps[:], ones_sb[:], bvrow[:], start=False, stop=True,
                            )
                        nc.vector.tensor_copy(
                            out=Vt[:, j, :, 0:64],
                            in_=ps.rearrange("p (h d) -> p h d", h=H),
                        )
                        nc.vector.memset(Vt[:, j, :, 64:65], 1.0)

                    if STAGE < 3:
                        continue
                    oT = act2.tile([P, ECH, 512], bf, tag="oT")
                    for s in range(2):      # the 2 sequences in this block
                        tb = s * 256        # col offset within the 512 block
                        probs = act2.tile([P, 2, H, 256], bf, tag="probs")
                        for st in range(2):  # s_tile (128 keys each)
                            tlo = 128 if st == 1 else 0
                            for hh in range(H):
                                c, off = divmod(hh * HS, P)
                                # each matmul gets its own offset-0 psum tile:
                                # outputs at nonzero tile offsets miscompute
                                # on HW (walrus bank mapping).
                                sc = ps_sc.tile([P, 256], f32, tag="sc", name="sc")
                                nc.tensor.matmul(
                                    sc[:, 0:256 - tlo],
                                    KT[off:off + HS, c, tb + st * P: tb + (st + 1) * P],
                                    QT[off:off + HS, c, tb + tlo: tb + 256],
                                    start=True, stop=True,
                                )
                                nc.scalar.activation(
                                    out=probs[:, st, hh, tlo:256],
                                    in_=sc[:, 0:256 - tlo],
                                    func=Act.Exp, scale=float(HS) ** -0.5,
                                )
                            if st == 0:
                                nc.vector.tensor_tensor(
                                    out=probs[:, 0], in0=probs[:, 0],
                                    in1=mask_sb[:, None, :].to_broadcast((P, H, 256)),
                                    op=Alu.mult,
                                )
                            else:
                                nc.vector.tensor_tensor(
                                    out=probs[:, 1, :, P:256],
                                    in0=probs[:, 1, :, P:256],
                                    in1=mask_sb[:, None, 0:P].to_broadcast((P, H, P)),
                                    op=Alu.mult,
                                )
                        if STAGE < 4:
                            continue
                        onorm = act2.tile([P, 2, E], bf, tag="onorm")
                        for tt in range(2):  # query tiles of this seq
                            # one single-shot matmul per (head, s-chunk), each
                            # into its own offset-0 psum tile; combine in SBUF.
                            osum = act2.tile([P, H, 65], f32, tag="osum")
                            for hh in range(H):
                                oa = ps_lin.tile([P, 65], f32, tag="mm", name="oa")
                                nc.tensor.matmul(
                                    oa[:],
                                    probs[:, 0, hh, tt * P:(tt + 1) * P],
                                    Vt[:, 2 * s, hh, :],
                                    start=True, stop=True,
                                )
                                nc.scalar.copy(out=osum[:, hh, :], in_=oa[:])
                                if tt == 1:
                                    oab = ps_lin.tile([P, 65], f32, tag="mm", name="oab")
                                    nc.tensor.matmul(
                                        oab[:],
                                        probs[:, 1, hh, P:2 * P],
                                        Vt[:, 2 * s + 1, hh, :],
                                        start=True, stop=True,
                                    )
                                    nc.vector.tensor_add(
                                        out=osum[:, hh, :], in0=osum[:, hh, :],
                                        in1=oab[:])
                            rec = act.tile([P, H], f32, tag="rec")
                            nc.vector.reciprocal(out=rec[:], in_=osum[:, :, 64])
                            nc.vector.tensor_tensor(
                                out=onorm[:, tt].rearrange("p (h d) -> p h d", h=H),
                                in0=osum[:, :, 0:64],
                                in1=rec[:, :, None].to_broadcast((P, H, HS)),
                                op=Alu.mult,
                            )
                        for tt in range(2):
                            for c in range(ECH):
                                tpose(
                                    oT[:, c, (2 * s + tt) * P:(2 * s + tt + 1) * P],
                                    onorm[:, tt, c * P:(c + 1) * P],
                                )
                    if STAGE < 5:
                        continue
                    linear_fmaj_resid(oT, wproj, ECH, bpc, i0, "p")

                    # --- MLP sublayer ---
                    if STAGE < 6:
                        continue
                    xnT2 = xnT if MLPVAR == "reuse" else ln_block(i0, "m")
                    aT = linear_fmaj(xnT2, w1, b1c, FCH, "aT",
                                     relu=(MLPVAR != "norelu"))
                    if MLPVAR == "w2dump":
                        nc.sync.dma_start(out=D["dbga"].ap(), in_=aT[:])
                        for j in range(4):
                            ps = ps_lin.tile([P, E], f32, tag="mm", name="psd")
                            for c in range(FCH):
                                lw = act.tile([P, P], bf, tag="lw", name="lw")
                                nc.vector.tensor_copy(
                                    out=lw[:], in_=aT[:, c, j * P:(j + 1) * P])
                                nc.tensor.matmul(
                                    ps[:], lw[:],
                                    w2[:, c, :],
                                    start=(c == 0), stop=(c == FCH - 1),
                                )
                            dtmp = act.tile([P, E], f32, tag="dtmp")
                            nc.vector.tensor_copy(out=dtmp[:], in_=ps[:])
                            nc.sync.dma_start(out=D["dbgo"].ap()[:, j, :], in_=dtmp[:])
                    elif MLPVAR == "dmastage":
                        aT2 = act1.tile([P, FCH, 512], bf, tag="aT2", name="aT2")
                        nc.sync.dma_start(out=aT2[:], in_=aT[:])
                        linear_fmaj_resid(aT2, w2, FCH, b2c, i0, "m")
                    elif MLPVAR != "w1only":
                        linear_fmaj_resid(aT, w2, FCH, b2c, i0, "m")

            # ---- final LN + unembed (feature-major logits) ----
            wout = wpool.tile([P, ECH, V], bf, tag="wout")
            nc.sync.dma_start(out=wout[:], in_=D["wout"].ap())
            for b in range(NB):
                xnfT = ln_block(4 * b, "f")
                ps = ps_lin.tile([V, 512], f32, tag="mm")
                for c in range(ECH):
                    nc.tensor.matmul(
                        ps[:], wout[:, c, :], xnfT[:, c, :],
                        start=(c == 0), stop=(c == ECH - 1),
                    )
                lt = act2.tile([V, 512], f32, tag="lt")
                nc.vector.tensor_scalar_add(out=lt[:], in0=ps[:], scalar1=boutc_sb[:])
                nc.sync.dma_start(
                    out=D["logT"].ap()[:, b * 512:(b + 1) * 512], in_=lt[:],
                )

    nc.compile()
    return nc


def _prep_shared(inp):
    """Host-side weight prep: layout rearrangement + LN gamma/beta folding."""
    sh = {}

    def f32(x):
        return np.asarray(x, np.float32)

    sh["te"] = np.asarray(f32(inp["tok_emb"]), BF16)                      # [V,E]
    sh["pos"] = np.ascontiguousarray(
        f32(inp["pos_emb"]).reshape(2, P, E).transpose(1, 0, 2))          # [P,2,E]
    m = np.concatenate(
        [np.triu(np.ones((P, P), np.float32)), np.ones((P, P), np.float32)], axis=1)
    sh["mask"] = np.asarray(m, BF16)                                      # [P,256]

    def tile3(w, fdim):  # [E, fdim] -> [P, ECH, fdim]
        return np.ascontiguousarray(w.reshape(ECH, P, fdim).transpose(1, 0, 2))

    def col(b, nch):  # [nch*P] -> [P, nch]
        return np.ascontiguousarray(b.reshape(nch, P).T)

    bv_nz, bp_nz, b2_nz = [], [], []
    for l in range(L):
        g1, b1_ = f32(inp["ln1_g"][l]), f32(inp["ln1_b"][l])
        g2, b2_ = f32(inp["ln2_g"][l]), f32(inp["ln2_b"][l])
        wq = f32(inp["Wq"][l]).transpose(1, 0, 2).reshape(E, E)   # head-major cols
        wk = f32(inp["Wk"][l]).transpose(1, 0, 2).reshape(E, E)
        wv = f32(inp["Wv"][l]).transpose(1, 0, 2).reshape(E, E)
        sh[f"wq{l}"] = np.asarray(tile3(g1[:, None] * wq, E), BF16)
        sh[f"wk{l}"] = np.asarray(tile3(g1[:, None] * wk, E), BF16)
        sh[f"wv{l}"] = np.asarray(tile3(g1[:, None] * wv, E), BF16)
        sh[f"bq{l}"] = col(wq.T @ b1_, ECH)
        sh[f"bk{l}"] = col(wk.T @ b1_, ECH)
        bv = wv.T @ b1_
        sh[f"bvrow{l}"] = np.asarray(bv[None, :], BF16)
        bv_nz.append(bool(np.any(bv != 0)))
        wp = f32(inp["Wproj"][l])
        sh[f"wproj{l}"] = np.asarray(tile3(wp, E), BF16)
        bp = f32(inp["bproj"][l])
        sh[f"bpc{l}"] = col(bp, ECH)
        bp_nz.append(bool(np.any(bp != 0)))
        w1 = f32(inp["W1"][l])
        sh[f"w1{l}"] = np.asarray(tile3(g2[:, None] * w1, FF), BF16)
        sh[f"b1c{l}"] = col(f32(inp["b1"][l]) + w1.T @ b2_, FCH)
        w2 = f32(inp["W2"][l])
        sh[f"w2{l}"] = np.asarray(
            w2.reshape(FCH, P, E).transpose(1, 0, 2), BF16)
        b2r = f32(inp["b2"][l])
        sh[f"b2c{l}"] = col(b2r, ECH)
        b2_nz.append(bool(np.any(b2r != 0)))

    gf, bf_ = f32(inp["lnf_g"]), f32(inp["lnf_b"])
    wo = f32(inp["Wout"])
    sh["wout"] = np.asarray(tile3(gf[:, None] * wo, V), BF16)
    sh["boutc"] = (f32(inp["bout"]) + wo.T @ bf_).reshape(V, 1)
    flags = (tuple(bv_nz), tuple(bp_nz), tuple(b2_nz))
    return sh, flags


def _onehot(xc):
    """xc: [BPC, T] ints -> [V, TOK] bf16 one-hot (feature-major)."""
    xf = np.asarray(xc, np.int64).reshape(-1)
    oh = np.zeros((V, TOK), np.float32)
    oh[xf, np.arange(TOK)] = 1.0
    return np.asarray(oh, BF16)


def _get_nc(flags):
    if flags not in _NC_CACHE:
        _NC_CACHE[flags] = _build_nc(flags)
    return _NC_CACHE[flags]


def make_in_maps(inputs):
    sh, flags = _prep_shared(inputs)
    x = np.asarray(inputs["x"])
    in_maps = []
    for c in range(NCORES):
        m = dict(sh)
        m["oh"] = _onehot(x[c * BPC:(c + 1) * BPC])
        in_maps.append(m)
    return in_maps, flags


def kernel(**inputs):
    import os
    from concourse.bass_utils import run_bass_kernel_spmd

    in_maps, flags = make_in_maps(inputs)
    nc = _get_nc(flags)
    kw = {}
    if os.environ.get("BASS_TRACE"):
        d = os.environ.get("BASS_TRACE_DIR", "/tmp/bass_trace")
        os.makedirs(d, exist_ok=True)
        kw["tmpdir"] = d
    res = run_bass_kernel_spmd(nc, in_maps, list(range(NCORES)), **kw)
    kernel._last = res
    outs = []
    for c in range(NCORES):
        lt = np.asarray(res.results[c]["logT"], np.float32)   # [V, TOK]
        outs.append(np.ascontiguousarray(lt.T).reshape(BPC, T, V))
    return np.concatenate(outs, axis=0)


kernel._last = None



# revision 5
# speedup vs baseline: 2.0505x; 2.0505x over previous
"""Trainium2 Bass kernel for a 3-layer dense transformer (BigramModel).

Contract: kernel(**inputs) takes the FULL unsharded numpy inputs (as produced
by setup_inputs) and returns the full [B, T, V] float32 logits. Internally the
batch dim B=128 is sharded 16-per-core across 8 NeuronCores (pure data
parallelism, weights replicated), one Bass/Tile NEFF run via
run_bass_kernel_spmd.

v2 design notes (vs the v1 baseline that was Sync-engine bound at 2.6ms):
  - All XBAR DMA transposes are batched 12-into-1: one dma_start_transpose per
    512-token block turns [128, 4, 384] token-major into [128, 12, 128]
    feature-major (c12 = j*3 + c). 1536 transpose instructions -> ~80.
  - Layers run as two passes (attention pass over all 8 blocks, then MLP
    pass) so LN stats + rstd are hoisted: rstd = reciprocal(sqrt(var+eps))
    costs one ACT Sqrt (table switch) + one DVE reciprocal per pass instead
    of Ln/Exp table thrash per block (real HW puts Ln and Exp in different
    ACT table sets; v1 paid 112 x 1.3us table loads).
  - proj and W2 run token-major (lhsT = transposed activations, rhs = W) so
    the residual add is one scalar_tensor_tensor from PSUM into h -- no
    reverse transposes, no separate bias/copy ops.
  - attention o accumulates both key-halves in one PSUM bank (start/stop),
    evacuated by per-head DVE reciprocal + tensor_scalar (no ACT copies).
  - softmax exp stays on ACT; W1 relu evac alternates ACT/DVE to balance.
"""

import numpy as np
import ml_dtypes

BF16 = ml_dtypes.bfloat16

P = 128
T = 256
E = 384
V = 65
H = 6
HS = 64
FF = 1536
L = 3
NCORES = 8
BPC = 16              # sequences per core
TOK = BPC * T         # 4096 tokens per core
NT = TOK // P         # 32 token tiles
NB = TOK // 512       # 8 blocks of 512 tokens (2 seqs)
ECH = E // P          # 3
FCH = FF // P         # 12

_NC_CACHE = {}


def _build_nc(flags):
    """Build + compile the Bass program.

    flags = (bq_nz, bk_nz, bv_nz, bp_nz, b1_nz, b2_nz, bout_nz) with per-layer
    tuples for the first six."""
    import concourse.bacc as bacc
    import concourse.mybir as mybir
    import concourse.tile as tile

    dt = mybir.dt
    f32 = dt.float32
    bf = dt.bfloat16
    Alu = mybir.AluOpType
    Act = mybir.ActivationFunctionType

    nc = bacc.Bacc("TRN2", target_bir_lowering=False, debug=False, num_devices=1)

    bq_nz, bk_nz, bv_nz, bp_nz, b1_nz, b2_nz, bout_nz = flags

    # ---- DRAM tensors ----
    D = {}
    D["oh"] = nc.dram_tensor("oh", [V, TOK], bf, kind="ExternalInput")
    D["te"] = nc.dram_tensor("te", [V, E], bf, kind="ExternalInput")
    D["pos"] = nc.dram_tensor("pos", [P, 2, E], f32, kind="ExternalInput")
    D["mask"] = nc.dram_tensor("mask", [P, P], bf, kind="ExternalInput")
    for l in range(L):
        for w in ("wq", "wk", "wv", "wproj"):
            D[f"{w}{l}"] = nc.dram_tensor(f"{w}{l}", [P, ECH, E], bf, kind="ExternalInput")
        D[f"w1{l}"] = nc.dram_tensor(f"w1{l}", [P, ECH, FF], bf, kind="ExternalInput")
        D[f"w2{l}"] = nc.dram_tensor(f"w2{l}", [P, FCH, E], bf, kind="ExternalInput")
        if bq_nz[l]:
            D[f"bq{l}"] = nc.dram_tensor(f"bq{l}", [P, ECH], f32, kind="ExternalInput")
        if bk_nz[l]:
            D[f"bk{l}"] = nc.dram_tensor(f"bk{l}", [P, ECH], f32, kind="ExternalInput")
        if bv_nz[l]:
            D[f"bvrow{l}"] = nc.dram_tensor(f"bvrow{l}", [1, E], bf, kind="ExternalInput")
        if bp_nz[l]:
            D[f"bprow{l}"] = nc.dram_tensor(f"bprow{l}", [1, E], bf, kind="ExternalInput")
        if b1_nz[l]:
            D[f"b1c{l}"] = nc.dram_tensor(f"b1c{l}", [P, FCH], f32, kind="ExternalInput")
        if b2_nz[l]:
            D[f"b2row{l}"] = nc.dram_tensor(f"b2row{l}", [1, E], bf, kind="ExternalInput")
    D["wout"] = nc.dram_tensor("wout", [P, ECH, V], bf, kind="ExternalInput")
    if bout_nz:
        D["boutc"] = nc.dram_tensor("boutc", [V, 1], f32, kind="ExternalInput")
    D["logT"] = nc.dram_tensor("logT", [V, TOK], f32, kind="ExternalOutput")

    with tile.TileContext(nc) as tc:
        import contextlib

        with contextlib.ExitStack() as ctx:
            const = ctx.enter_context(tc.tile_pool(name="const", bufs=1))
            wpool = ctx.enter_context(tc.tile_pool(name="wpool", bufs=2))
            act = ctx.enter_context(tc.tile_pool(name="act", bufs=2))
            acts = ctx.enter_context(tc.tile_pool(name="acts", bufs=3))
            act1 = ctx.enter_context(tc.tile_pool(name="act1", bufs=1))
            ps_lin = ctx.enter_context(tc.tile_pool(name="ps_lin", bufs=4, space="PSUM"))
            ps_sc = ctx.enter_context(tc.tile_pool(name="ps_sc", bufs=2, space="PSUM"))
            ps_o = ctx.enter_context(tc.tile_pool(name="ps_o", bufs=2, space="PSUM"))

            # ---- constants ----
            # pad the K=65 embedding contraction to K=128 (sub-128 partition
            # matmuls are flaky on HW); pad rows are zeroed so they add 0.
            oh_sb = const.tile([P, TOK], bf, tag="oh")
            nc.vector.memset(oh_sb[:], 0.0)
            nc.sync.dma_start(out=oh_sb[0:V, :], in_=D["oh"].ap())
            te_sb = const.tile([P, E], bf, tag="te")
            nc.vector.memset(te_sb[:], 0.0)
            nc.sync.dma_start(out=te_sb[0:V, :], in_=D["te"].ap())
            pos_sb = const.tile([P, 2, E], f32, tag="pos")
            nc.sync.dma_start(out=pos_sb[:], in_=D["pos"].ap())
            mask_sb = const.tile([P, P], bf, tag="mask")
            nc.sync.dma_start(out=mask_sb[:], in_=D["mask"].ap())
            boutc_sb = None
            if bout_nz:
                boutc_sb = const.tile([V, 1], f32, tag="boutc")
                nc.sync.dma_start(out=boutc_sb[:], in_=D["boutc"].ap())
            ones_sb = const.tile([1, P], bf, tag="ones")
            nc.vector.memset(ones_sb[:], 1.0)
            eps_sb = const.tile([P, 1], f32, tag="eps")
            nc.vector.memset(eps_sb[:], 1e-5)
            zero_sb = const.tile([P, 1], f32, tag="zero")
            nc.vector.memset(zero_sb[:], 0.0)

            # persistent residual tiles (token-major fp32)
            h = [const.tile([P, E], f32, tag=f"h{i}", name=f"h{i}") for i in range(NT)]

            # ---- embedding: h = onehot.T @ tok_emb + pos ----
            for i in range(NT):
                ps = ps_lin.tile([P, 512], f32, tag="mm")
                nc.tensor.matmul(
                    ps[:, 0:E], oh_sb[:, i * P:(i + 1) * P], te_sb[:],
                    start=True, stop=True,
                )
                nc.vector.tensor_add(out=h[i][:], in0=ps[:, 0:E], in1=pos_sb[:, i % 2, :])

            # ---- LN helpers ----
            def ln_stats(tag):
                """bn_stats over all 32 h tiles -> (mv [P,NT,2], rstd [P,NT])."""
                mv = act.tile([P, NT, 2], f32, tag=f"mv_{tag}", name=f"mv_{tag}")
                for i in range(NT):
                    st6 = acts.tile([P, 6], f32, tag="bnst")
                    nc.vector.bn_stats(out=st6[:], in_=h[i][:])
                    nc.vector.bn_aggr(out=mv[:, i, :], in_=st6[:])
                sd = act.tile([P, NT], f32, tag=f"sd_{tag}", name=f"sd_{tag}")
                nc.scalar.activation(
                    out=sd[:], in_=mv[:, :, 1], func=Act.Sqrt, bias=eps_sb[:],
                )
                rstd = act.tile([P, NT], f32, tag=f"rs_{tag}", name=f"rs_{tag}")
                nc.vector.reciprocal(out=rstd[:], in_=sd[:])
                return mv, rstd

            def make_xnT(i0, mv, rstd):
                """xn = (h - m) * rstd for 4 tiles -> single batched transpose
                to feature-major [P, 12, 128] (c12 = j*3 + c)."""
                xn4 = act.tile([P, 4, E], bf, tag="xn4")
                for j in range(4):
                    nc.vector.tensor_scalar(
                        out=xn4[:, j, :], in0=h[i0 + j][:],
                        scalar1=mv[:, i0 + j, 0:1], scalar2=rstd[:, i0 + j:i0 + j + 1],
                        op0=Alu.subtract, op1=Alu.mult,
                    )
                xnT = act.tile([P, 12, P], bf, tag="xnT")
                nc.sync.dma_start_transpose(
                    xnT[:], xn4[:].rearrange("p a b -> p (a b)"))
                return xnT

            def lin_fmaj(xnT, w_sb, bias_col, fch, tag, evac, pool=None):
                """feature-major out [P, fch, 512] bf16; evac in {dve, act, mixN}."""
                o = (pool or act).tile([P, fch, 512], bf, tag=tag, name=tag)
                rhs_view = xnT[:].rearrange("p (j c) a -> p c j a", c=ECH)
                for f in range(fch):
                    ps = ps_lin.tile([P, 512], f32, tag="mm")
                    for c in range(ECH):
                        nc.tensor.matmul(
                            ps[:], w_sb[:, c, f * P:(f + 1) * P], rhs_view[:, c],
                            start=(c == 0), stop=(c == ECH - 1),
                        )
                    use_act = (evac == "act") or (evac == "mix" and f % 2 == 0)
                    if use_act:
                        if bias_col is not None:
                            nc.scalar.activation(
                                out=o[:, f, :], in_=ps[:], func=Act.Copy,
                                bias=bias_col[:, f:f + 1])
                        else:
                            nc.scalar.copy(out=o[:, f, :], in_=ps[:])
                    else:
                        if bias_col is not None:
                            nc.vector.tensor_scalar_add(
                                out=o[:, f, :], in0=ps[:],
                                scalar1=bias_col[:, f:f + 1])
                        else:
                            nc.vector.tensor_copy(out=o[:, f, :], in_=ps[:])
                return o

            def lin_fmaj_relu(xnT, w_sb, bias_col, tag):
                """W1 + relu, evac alternating ACT/DVE."""
                o = act1.tile([P, FCH, 512], bf, tag=tag, name=tag)
                rhs_view = xnT[:].rearrange("p (j c) a -> p c j a", c=ECH)
                for f in range(FCH):
                    ps = ps_lin.tile([P, 512], f32, tag="mm")
                    for c in range(ECH):
                        nc.tensor.matmul(
                            ps[:], w_sb[:, c, f * P:(f + 1) * P], rhs_view[:, c],
                            start=(c == 0), stop=(c == ECH - 1),
                        )
                    if f % 2 == 0:
                        nc.scalar.activation(
                            out=o[:, f, :], in_=ps[:], func=Act.Relu,
                            bias=(bias_col[:, f:f + 1] if bias_col is not None else 0.0))
                    else:
                        if bias_col is not None:
                            nc.vector.tensor_scalar(
                                out=o[:, f, :], in0=ps[:],
                                scalar1=bias_col[:, f:f + 1], scalar2=zero_sb[:],
                                op0=Alu.add, op1=Alu.max,
                            )
                        else:
                            nc.vector.tensor_scalar_max(
                                out=o[:, f, :], in0=ps[:], scalar1=zero_sb[:],
                            )
                return o

            def lin_tmaj_resid(xT, w_sb, nch, brow, i0):
                """h[i0+j] += xT_j @ W + brow, token-major: one STT per tile."""
                for j in range(4):
                    ps = ps_lin.tile([P, 512], f32, tag="mm")
                    for c in range(nch):
                        nc.tensor.matmul(
                            ps[:, 0:E], xT[:, j * nch + c, :] if nch == ECH
                            else xT[:, c, j * P:(j + 1) * P],
                            w_sb[:, c, :],
                            start=(c == 0),
                            stop=(c == nch - 1 and brow is None),
                        )
                    if brow is not None:
                        nc.tensor.matmul(
                            ps[:, 0:E], ones_sb[:], brow[:], start=False, stop=True,
                        )
                    nc.vector.scalar_tensor_tensor(
                        out=h[i0 + j][:], in0=ps[:, 0:E], scalar=0.0,
                        in1=h[i0 + j][:], op0=Alu.add, op1=Alu.add,
                    )

            def load_w(name, shape, dtp, tag=None):
                t = wpool.tile(shape, dtp, tag=tag or name[:-1])
                nc.sync.dma_start(out=t[:], in_=D[name].ap())
                return t

            scale = float(HS) ** -0.5

            # ---- transformer layers ----
            for l in range(L):
                wq = load_w(f"wq{l}", [P, ECH, E], bf)
                wk = load_w(f"wk{l}", [P, ECH, E], bf)
                wv = load_w(f"wv{l}", [P, ECH, E], bf)
                wproj = load_w(f"wproj{l}", [P, ECH, E], bf)
                w1 = load_w(f"w1{l}", [P, ECH, FF], bf)
                w2 = load_w(f"w2{l}", [P, FCH, E], bf)
                bq = load_w(f"bq{l}", [P, ECH], f32) if bq_nz[l] else None
                bk = load_w(f"bk{l}", [P, ECH], f32) if bk_nz[l] else None
                bvrow = load_w(f"bvrow{l}", [1, E], bf) if bv_nz[l] else None
                bprow = load_w(f"bprow{l}", [1, E], bf) if bp_nz[l] else None
                b1c = load_w(f"b1c{l}", [P, FCH], f32) if b1_nz[l] else None
                b2row = load_w(f"b2row{l}", [1, E], bf) if b2_nz[l] else None

                # ======== attention pass ========
                mv1, rstd1 = ln_stats(f"a{l}")
                for b in range(NB):
                    i0 = 4 * b
                    xnT = make_xnT(i0, mv1, rstd1)
                    QT = lin_fmaj(xnT, wq, bq, ECH, "QT", "dve")
                    KT = lin_fmaj(xnT, wk, bk, ECH, "KT", "act")
                    # V token-major, ones-augmented: [P, 4, H, 65]
                    Vt = act.tile([P, 4, H, 65], bf, tag="Vt")
                    for j in range(4):
                        ps = ps_lin.tile([P, 512], f32, tag="mm")
                        for c in range(ECH):
                            nc.tensor.matmul(
                                ps[:, 0:E], xnT[:, j * ECH + c, :], wv[:, c, :],
                                start=(c == 0),
                                stop=(c == ECH - 1 and bvrow is None),
                            )
                        if bvrow is not None:
                            nc.tensor.matmul(
                                ps[:, 0:E], ones_sb[:], bvrow[:],
                                start=False, stop=True,
                            )
                        nc.vector.tensor_copy(
                            out=Vt[:, j, :, 0:64],
                            in_=ps[:, 0:E].rearrange("p (h d) -> p h d", h=H),
                        )
                        nc.vector.memset(Vt[:, j, :, 64:65], 1.0)

                    onorm4 = act.tile([P, 4, E], bf, tag="onorm4")
                    for s in range(2):      # the 2 sequences in this block
                        tb = s * 256        # col offset within the 512 block
                        probs = acts.tile([P, 2, H, 256], bf, tag="probs")
                        for st in range(2):  # key tile (128 keys each)
                            tlo = 128 if st == 1 else 0
                            for hh in range(H):
                                c, off = divmod(hh * HS, P)
                                sc = ps_sc.tile([P, 256], f32, tag="sc", name="sc")
                                nc.tensor.matmul(
                                    sc[:, 0:256 - tlo],
                                    KT[off:off + HS, c, tb + st * P: tb + (st + 1) * P],
                                    QT[off:off + HS, c, tb + tlo: tb + 256],
                                    start=True, stop=True,
                                )
                                nc.scalar.activation(
                                    out=probs[:, st, hh, tlo:256],
                                    in_=sc[:, 0:256 - tlo],
                                    func=Act.Exp, scale=scale,
                                )
                            # causal mask: only the diagonal 128x128 needs it
                            nc.vector.tensor_tensor(
                                out=probs[:, st, :, tlo:tlo + P],
                                in0=probs[:, st, :, tlo:tlo + P],
                                in1=mask_sb[:, None, :].to_broadcast((P, H, P)),
                                op=Alu.mult,
                            )
                        for tt in range(2):  # query tiles of this seq
                            for hh in range(H):
                                po = ps_o.tile([P, 65], f32, tag="po", name="po")
                                if tt == 0:
                                    nc.tensor.matmul(
                                        po[:], probs[:, 0, hh, 0:P],
                                        Vt[:, 2 * s, hh, :],
                                        start=True, stop=True,
                                    )
                                else:
                                    nc.tensor.matmul(
                                        po[:], probs[:, 0, hh, P:256],
                                        Vt[:, 2 * s, hh, :],
                                        start=True, stop=False,
                                    )
                                    nc.tensor.matmul(
                                        po[:], probs[:, 1, hh, P:256],
                                        Vt[:, 2 * s + 1, hh, :],
                                        start=False, stop=True,
                                    )
                                rec = acts.tile([P, 1], f32, tag="rec")
                                nc.vector.reciprocal(out=rec[:], in_=po[:, 64:65])
                                nc.vector.tensor_scalar_mul(
                                    out=onorm4[:, 2 * s + tt, hh * 64:(hh + 1) * 64],
                                    in0=po[:, 0:64], scalar1=rec[:, 0:1],
                                )
                    oT = act.tile([P, 12, P], bf, tag="oT")
                    nc.sync.dma_start_transpose(
                        oT[:], onorm4[:].rearrange("p a b -> p (a b)"))
                    lin_tmaj_resid(oT, wproj, ECH, bprow, i0)

                # ======== MLP pass ========
                mv2, rstd2 = ln_stats(f"m{l}")
                for b in range(NB):
                    i0 = 4 * b
                    xnT = make_xnT(i0, mv2, rstd2)
                    aT = lin_fmaj_relu(xnT, w1, b1c, "aT")
                    lin_tmaj_resid(aT, w2, FCH, b2row, i0)

            # ---- final LN + unembed (feature-major logits) ----
            wout = wpool.tile([P, ECH, V], bf, tag="wout")
            nc.sync.dma_start(out=wout[:], in_=D["wout"].ap())
            mvf, rstdf = ln_stats("f")
            for b in range(NB):
                xnT = make_xnT(4 * b, mvf, rstdf)
                rhs_view = xnT[:].rearrange("p (j c) a -> p c j a", c=ECH)
                ps = ps_lin.tile([V, 512], f32, tag="mm", name="mmv")
                for c in range(ECH):
                    nc.tensor.matmul(
                        ps[:], wout[:, c, :], rhs_view[:, c],
                        start=(c == 0), stop=(c == ECH - 1),
                    )
                lt = acts.tile([V, 512], f32, tag="lt")
                if boutc_sb is not None:
                    nc.vector.tensor_scalar_add(
                        out=lt[:], in0=ps[:], scalar1=boutc_sb[:])
                else:
                    nc.vector.tensor_copy(out=lt[:], in_=ps[:])
                nc.sync.dma_start(
                    out=D["logT"].ap()[:, b * 512:(b + 1) * 512], in_=lt[:],
                )

    nc.compile()
    return nc


def _prep_shared(inp):
    """Host-side weight prep: layout rearrangement + LN gamma/beta folding."""
    sh = {}

    def f32(x):
        return np.asarray(x, np.float32)

    sh["te"] = np.asarray(f32(inp["tok_emb"]), BF16)                      # [V,E]
    sh["pos"] = np.ascontiguousarray(
        f32(inp["pos_emb"]).reshape(2, P, E).transpose(1, 0, 2))          # [P,2,E]
    sh["mask"] = np.asarray(np.triu(np.ones((P, P), np.float32)), BF16)   # [P,P]

    def tile3(w, fdim):  # [E, fdim] -> [P, ECH, fdim]
        return np.ascontiguousarray(w.reshape(ECH, P, fdim).transpose(1, 0, 2))

    def col(b, nch):  # [nch*P] -> [P, nch]
        return np.ascontiguousarray(b.reshape(nch, P).T)

    bq_nz, bk_nz, bv_nz, bp_nz, b1_nz, b2_nz = [], [], [], [], [], []
    for l in range(L):
        g1, b1_ = f32(inp["ln1_g"][l]), f32(inp["ln1_b"][l])
        g2, b2_ = f32(inp["ln2_g"][l]), f32(inp["ln2_b"][l])
        wq = f32(inp["Wq"][l]).transpose(1, 0, 2).reshape(E, E)   # head-major cols
        wk = f32(inp["Wk"][l]).transpose(1, 0, 2).reshape(E, E)
        wv = f32(inp["Wv"][l]).transpose(1, 0, 2).reshape(E, E)
        sh[f"wq{l}"] = np.asarray(tile3(g1[:, None] * wq, E), BF16)
        sh[f"wk{l}"] = np.asarray(tile3(g1[:, None] * wk, E), BF16)
        sh[f"wv{l}"] = np.asarray(tile3(g1[:, None] * wv, E), BF16)
        bq = wq.T @ b1_
        bk = wk.T @ b1_
        bv = wv.T @ b1_
        bq_nz.append(bool(np.any(bq != 0)))
        bk_nz.append(bool(np.any(bk != 0)))
        bv_nz.append(bool(np.any(bv != 0)))
        if bq_nz[-1]:
            sh[f"bq{l}"] = col(bq, ECH)
        if bk_nz[-1]:
            sh[f"bk{l}"] = col(bk, ECH)
        if bv_nz[-1]:
            sh[f"bvrow{l}"] = np.asarray(bv[None, :], BF16)
        wp = f32(inp["Wproj"][l])
        sh[f"wproj{l}"] = np.asarray(tile3(wp, E), BF16)
        bp = f32(inp["bproj"][l])
        bp_nz.append(bool(np.any(bp != 0)))
        if bp_nz[-1]:
            sh[f"bprow{l}"] = np.asarray(bp[None, :], BF16)
        w1 = f32(inp["W1"][l])
        sh[f"w1{l}"] = np.asarray(tile3(g2[:, None] * w1, FF), BF16)
        b1ff = f32(inp["b1"][l]) + w1.T @ b2_
        b1_nz.append(bool(np.any(b1ff != 0)))
        if b1_nz[-1]:
            sh[f"b1c{l}"] = col(b1ff, FCH)
        w2 = f32(inp["W2"][l])
        sh[f"w2{l}"] = np.asarray(
            w2.reshape(FCH, P, E).transpose(1, 0, 2), BF16)
        b2r = f32(inp["b2"][l])
        b2_nz.append(bool(np.any(b2r != 0)))
        if b2_nz[-1]:
            sh[f"b2row{l}"] = np.asarray(b2r[None, :], BF16)

    gf, bf_ = f32(inp["lnf_g"]), f32(inp["lnf_b"])
    wo = f32(inp["Wout"])
    sh["wout"] = np.asarray(tile3(gf[:, None] * wo, V), BF16)
    boutc = f32(inp["bout"]) + wo.T @ bf_
    bout_nz = bool(np.any(boutc != 0))
    if bout_nz:
        sh["boutc"] = boutc.reshape(V, 1)
    flags = (tuple(bq_nz), tuple(bk_nz), tuple(bv_nz), tuple(bp_nz),
             tuple(b1_nz), tuple(b2_nz), bout_nz)
    return sh, flags


def _onehot(xc):
    """xc: [BPC, T] ints -> [V, TOK] bf16 one-hot (feature-major)."""
    xf = np.asarray(xc, np.int64).reshape(-1)
    oh = np.zeros((V, TOK), np.float32)
    oh[xf, np.arange(TOK)] = 1.0
    return np.asarray(oh, BF16)


def _get_nc(flags):
    if flags not in _NC_CACHE:
        _NC_CACHE[flags] = _build_nc(flags)
    return _NC_CACHE[flags]


def make_in_maps(inputs):
    sh, flags = _prep_shared(inputs)
    x = np.asarray(inputs["x"])
    in_maps = []
    for c in range(NCORES):
        m = dict(sh)
        m["oh"] = _onehot(x[c * BPC:(c + 1) * BPC])
        in_maps.append(m)
    return in_maps, flags


def kernel(**inputs):
    import os
    from concourse.bass_utils import run_bass_kernel_spmd

    in_maps, flags = make_in_maps(inputs)
    nc = _get_nc(flags)
    kw = {}
    if os.environ.get("BASS_TRACE"):
        d = os.environ.get("BASS_TRACE_DIR", "/tmp/bass_trace")
        os.makedirs(d, exist_ok=True)
        kw["tmpdir"] = d
    res = run_bass_kernel_spmd(nc, in_maps, list(range(NCORES)), **kw)
    kernel._last = res
    outs = []
    for c in range(NCORES):
        lt = np.asarray(res.results[c]["logT"], np.float32)   # [V, TOK]
        outs.append(np.ascontiguousarray(lt.T).reshape(BPC, T, V))
    return np.concatenate(outs, axis=0)


kernel._last = None
